# revision 42
# baseline (speedup 1.0000x reference)
"""GCNGraphDTA Trainium2 kernel.

Strategy: graphs are independent 25-node blocks, so each GCN layer
    h' = relu( D^-1/2 (A+I) D^-1/2 (h W) + b )
is dense linear algebra with a block-diagonal normalized adjacency.
On the host (sharding step) we build, per graph, the 25x25 matrix
    AT_g[u, v] = dinv[u] * dinv[v] * count(u->v) + dinv[u]^2 * delta_uv
(the transpose of the propagation matrix), pack 5 graphs into a 125x125
block-diagonal tile, and hand each of the 8 cores its 256 graphs
(padded to 260 = 52 tiles) plus replicated weights.

On device, per layer and per batch of 8 groups (two PSUM banks):
  - 8 matmuls  hW: out[node, f128] = H_fm[:, cols].T @ W           (PSUM)
  - 2 half-casts PSUM->SBUF of the [128, 1024] batch to fp16 (DVE + ACT)
  - 8 matmuls agg: out[f128, node125] = hW_nm.T @ AT_tile          (PSUM)
  - 1 fused relu(x + b) PSUM->SBUF into H_fm of the next layer
with a 1-deep software pipeline (batch b's aggs issue behind batch
b+1's hW matmuls) so the PE never waits on the casts.  All matmul
operands are fp16 (PSUM accumulates fp32); a dummy-matmul warm-up burst
during the input DMA pushes the PE HAM clock-gate to 8/8 before the
real stream.  H tensors use a 128-wide per-group column stride so each
agg matmul output (N=125) stays inside one PSUM bank.

Then global max pool = 4 chunked DVE reduce_max over 25-wide windows
interleaved into layer 3, and the [256,256]x[256,1] MLP as 4+2 matmuls.
"""

import numpy as np

import concourse.bass as bass
import concourse.mybir as mybir
import concourse.tile as tile
from concourse.bass_utils import run_bass_kernel_spmd

N_CORES = 8
N_GRAPHS = 2048
NPG = 25               # nodes per graph
N_NODES = N_GRAPHS * NPG
F_IN = 13
HID = 128
PROT = 128
GPC = N_GRAPHS // N_CORES      # 256 graphs per core
PAD_G = 260                    # padded to a multiple of 5
GPG = 5                        # graphs per 125-row group
GROUPS = PAD_G // GPG          # 52
GW = GPG * NPG                 # 125 = group width (nodes)
GS = 128                       # group column stride in H layout (PSUM bank align)
COLS_A = GROUPS * GW           # 6500: AT columns (dense 125-wide groups)
COLS_H = GROUPS * GS           # 6656: H/xT columns (128-wide groups, 3 dead)
BATCH = 8                      # groups per PSUM batch (2 banks)
N_BATCH = (GROUPS + BATCH - 1) // BATCH  # 7 (last batch has 4 groups)
N_WARM = 10                    # dummy matmuls to warm the PE clock gate

F32 = mybir.dt.float32
F16 = mybir.dt.float16


def _split_multi_waits(nc):
    """This container's walrus build accepts at most one sem wait per
    instruction (two for EventSemaphore). Tile emits multi-waits freely, so
    hoist the extras onto same-engine NoOps inserted just before."""
    for f in nc.m.functions:
        for blk in f.blocks:
            new_insts = []
            for inst in blk.instructions:
                si = getattr(inst, "sync_info", None)
                cap = 2 if inst.opcode == "EventSemaphore" else 1
                if si is not None and si.on_wait and len(si.on_wait) > cap:
                    waits = list(si.on_wait)
                    for i, w in enumerate(waits[:-cap]):
                        new_insts.append(mybir.InstNoOp(
                            name=f"{inst.name}-ws{i}",
                            engine=inst.engine,
                            bass_nofuse=True,
                            sync_info=mybir.SyncInfo(on_wait=[w], on_update=[]),
                        ))
                    si.on_wait = waits[-cap:]
                new_insts.append(inst)
            blk.instructions[:] = new_insts


def _build_program():
    nc = bass.Bass()

    xT = nc.dram_tensor("xT", [F_IN, COLS_H], F16, kind="ExternalInput")
    AT = nc.dram_tensor("AT", [GW, COLS_A], F16, kind="ExternalInput")
    W1 = nc.dram_tensor("W1", [F_IN, HID], F16, kind="ExternalInput")
    W2 = nc.dram_tensor("W2", [HID, HID], F16, kind="ExternalInput")
    W3 = nc.dram_tensor("W3", [HID, HID], F16, kind="ExternalInput")
    B1 = nc.dram_tensor("B1", [HID, 1], F32, kind="ExternalInput")
    B2 = nc.dram_tensor("B2", [HID, 1], F32, kind="ExternalInput")
    B3 = nc.dram_tensor("B3", [HID, 1], F32, kind="ExternalInput")
    WF1 = nc.dram_tensor("WF1", [2 * HID, 256], F16, kind="ExternalInput")
    BF1 = nc.dram_tensor("BF1", [256, 1], F32, kind="ExternalInput")
    WF2 = nc.dram_tensor("WF2", [256, 1], F16, kind="ExternalInput")
    BF2 = nc.dram_tensor("BF2", [1, 1], F32, kind="ExternalInput")
    PT = nc.dram_tensor("PT", [PROT, GPC], F16, kind="ExternalInput")
    OUT = nc.dram_tensor("out", [1, GPC], F32, kind="ExternalOutput")

    with tile.TileContext(nc) as tc:
        with (
            tc.tile_pool(name="const", bufs=1) as cpool,
            tc.tile_pool(name="hw", bufs=3) as hwpool,
            tc.tile_pool(name="psum", bufs=2, space="PSUM") as pspool,
        ):  # psum: "mm" 2x2banks + "agg" 2x2banks = 8 banks
            # ---- persistent SBUF tensors ----
            w1_sb = cpool.tile([F_IN, HID], F16)
            w2_sb = cpool.tile([HID, HID], F16)
            w3_sb = cpool.tile([HID, HID], F16)
            b1_sb = cpool.tile([HID, 1], F32)
            b2_sb = cpool.tile([HID, 1], F32)
            b3_sb = cpool.tile([HID, 1], F32)
            wf1a_sb = cpool.tile([HID, 256], F16)   # Wf1 rows 0..127 (drug)
            wf1b_sb = cpool.tile([HID, 256], F16)   # Wf1 rows 128..255 (prot)
            bf1a_sb = cpool.tile([HID, 1], F32)
            bf1b_sb = cpool.tile([HID, 1], F32)
            wf2a_sb = cpool.tile([HID, 1], F16)
            wf2b_sb = cpool.tile([HID, 1], F16)
            bf2_sb = cpool.tile([1, 1], F32)
            pt_sb = cpool.tile([PROT, GPC], F16)
            xT_sb = cpool.tile([F_IN, COLS_H], F16)
            at_sb = cpool.tile([GW, COLS_A], F16)
            h1_sb = cpool.tile([HID, COLS_H], F16)
            h2_sb = cpool.tile([HID, COLS_H], F16)
            h3_sb = cpool.tile([HID, COLS_H], F16)
            drug_sb = cpool.tile([HID, PAD_G], F16)
            fc1a_sb = cpool.tile([HID, GPC], F16)
            fc1b_sb = cpool.tile([HID, GPC], F16)
            out_sb = cpool.tile([1, GPC], F32)
            warm_a = cpool.tile([HID, HID], F16)
            warm_b = cpool.tile([HID, 512], F16)

            # ---- PE warm-up: dense dummy matmuls during the DMA head so the
            # HAM clock gate reaches 8/8 before the real stream begins ----
            nc.gpsimd.memset(warm_a[:], 0.0)
            nc.gpsimd.memset(warm_b[:], 0.0)
            for i in range(N_WARM):
                warm_ps = pspool.tile([HID, 512], F32, tag="mm", name="warm_ps")
                nc.tensor.matmul(out=warm_ps[:], lhsT=warm_a[:], rhs=warm_b[:],
                                 start=True, stop=True)

            # ---- input DMAs, in need-order per queue: the layer-1 critical
            # path (W1, xT) rides the otherwise-idle scalar queue; AT chunks
            # round-robin sync/gpsimd/scalar so chunk c lands ~in consumption
            # order ----
            nc.scalar.dma_start(out=w1_sb[:], in_=W1[:])
            nc.scalar.dma_start(out=b1_sb[:], in_=B1[:])
            nc.scalar.dma_start(out=xT_sb[:], in_=xT[:])
            n_chunk = 6
            ab = [COLS_A * c // n_chunk // GW * GW for c in range(n_chunk)] + [COLS_A]
            at_q = [nc.sync, nc.gpsimd, nc.scalar,
                    nc.sync, nc.gpsimd, nc.scalar]
            for c in range(n_chunk):
                at_q[c].dma_start(out=at_sb[:, ab[c]:ab[c + 1]],
                                  in_=AT[:, ab[c]:ab[c + 1]])
            nc.sync.dma_start(out=w2_sb[:], in_=W2[:])
            nc.sync.dma_start(out=b2_sb[:], in_=B2[:])
            nc.sync.dma_start(out=w3_sb[:], in_=W3[:])
            nc.sync.dma_start(out=b3_sb[:], in_=B3[:])
            nc.gpsimd.dma_start(out=wf1a_sb[:], in_=WF1[0:HID, :])
            nc.gpsimd.dma_start(out=wf1b_sb[:], in_=WF1[HID:2 * HID, :])
            nc.gpsimd.dma_start(out=bf1a_sb[:], in_=BF1[0:HID, :])
            nc.gpsimd.dma_start(out=bf1b_sb[:], in_=BF1[HID:256, :])
            nc.gpsimd.dma_start(out=wf2a_sb[:], in_=WF2[0:HID, :])
            nc.gpsimd.dma_start(out=wf2b_sb[:], in_=WF2[HID:256, :])
            nc.gpsimd.dma_start(out=bf2_sb[:], in_=BF2[:])
            nc.gpsimd.dma_start(out=pt_sb[:], in_=PT[:])

            # ---- 3 GCN layers ----
            layers = [
                (xT_sb, w1_sb, b1_sb, h1_sb),
                (h1_sb, w2_sb, b2_sb, h2_sb),
                (h2_sb, w3_sb, b3_sb, h3_sb),
            ]
            relu = mybir.ActivationFunctionType.Relu
            # global max pool runs as 4 chunked reduces interleaved into
            # layer 3 (chunk c is ready once batch POOL_AFTER[c] is done)
            PGROUPS = GROUPS // 4          # 13 groups per pool chunk
            PGRAPH = PGROUPS * GPG         # 65 graphs per pool chunk
            POOL_AFTER = {1: 0, 3: 1, 4: 2, 6: 3}

            def emit_pool(c):
                cols = slice(c * PGROUPS * GS, (c + 1) * PGROUPS * GS)
                view = (h3_sb[:, cols]
                        .rearrange("p (g c2) -> p g c2", c2=GS)[:, :, 0:GW]
                        .rearrange("p g (j n) -> p g j n", n=NPG))
                nc.vector.reduce_max(
                    out=drug_sb[:, c * PGRAPH:(c + 1) * PGRAPH],
                    in_=view, axis=mybir.AxisListType.X,
                )

            for li, (h_in, w_sb, b_sb, h_out) in enumerate(layers):

                def emit_agg(b, groups, hw_sb):
                    # second pipeline stage for batch b: agg matmuls + relu
                    nb = len(groups)
                    agg_ps = pspool.tile([HID, nb * GS], F32, tag="agg",
                                         name="agg_ps")
                    for gi, g in enumerate(groups):
                        nc.tensor.matmul(
                            out=agg_ps[:, gi * GS:gi * GS + GW],
                            lhsT=hw_sb[0:GW, gi * HID:(gi + 1) * HID],
                            rhs=at_sb[:, g * GW:(g + 1) * GW],
                            start=True, stop=True,
                        )
                    h_slice = h_out[:, groups[0] * GS:groups[0] * GS + nb * GS]
                    if (li * N_BATCH + b) % 3 == 2:
                        nc.vector.tensor_scalar(
                            out=h_slice, in0=agg_ps[:],
                            scalar1=b_sb[:], scalar2=0.0,
                            op0=mybir.AluOpType.add, op1=mybir.AluOpType.max,
                        )
                    else:
                        nc.scalar.activation(out=h_slice, in_=agg_ps[:],
                                             func=relu, bias=b_sb[:])
                    if li == 2 and b in POOL_AFTER:
                        emit_pool(POOL_AFTER[b])

                pend = None
                for b in range(N_BATCH):
                    groups = list(range(b * BATCH, min(GROUPS, (b + 1) * BATCH)))
                    nb = len(groups)
                    hw_ps = pspool.tile([HID, nb * HID], F32, tag="mm")
                    for gi, g in enumerate(groups):
                        nc.tensor.matmul(
                            out=hw_ps[:, gi * HID:(gi + 1) * HID],
                            lhsT=h_in[:, g * GS:(g + 1) * GS],
                            rhs=w_sb[:],
                            start=True, stop=True,
                        )
                    # PSUM->SBUF fp16 cast, split in two so the agg matmuls
                    # of this batch can start after the first half
                    hw_sb = hwpool.tile([HID, nb * HID], F16)
                    half = nb * HID // 2
                    nc.vector.tensor_copy(out=hw_sb[:, 0:half],
                                          in_=hw_ps[:, 0:half])
                    nc.scalar.copy(out=hw_sb[:, half:nb * HID],
                                   in_=hw_ps[:, half:nb * HID])
                    # 1-deep software pipeline: previous batch's agg+relu
                    # issue behind this batch's hW matmuls, so the PE never
                    # waits on the cast
                    if pend is not None:
                        emit_agg(*pend)
                    pend = (b, groups, hw_sb)
                emit_agg(*pend)

            # ---- MLP: relu([drug; prot] @ Wf1 + bf1) @ Wf2 + bf2 ----
            for mc, (fc1_sb, bf1_sb) in enumerate(
                    [(fc1a_sb, bf1a_sb), (fc1b_sb, bf1b_sb)]):
                fc1_ps = pspool.tile([HID, GPC], F32, tag="mm", name=f"fc1_ps_{mc}")
                ms = slice(mc * HID, (mc + 1) * HID)
                nc.tensor.matmul(out=fc1_ps[:], lhsT=wf1a_sb[:, ms],
                                 rhs=drug_sb[:, 0:GPC], start=True, stop=False)
                nc.tensor.matmul(out=fc1_ps[:], lhsT=wf1b_sb[:, ms],
                                 rhs=pt_sb[:], start=False, stop=True)
                nc.scalar.activation(out=fc1_sb[:], in_=fc1_ps[:],
                                     func=relu, bias=bf1_sb[:])
            fc2_ps = pspool.tile([1, GPC], F32, tag="agg", name="fc2_ps")
            nc.tensor.matmul(out=fc2_ps[:], lhsT=wf2a_sb[:], rhs=fc1a_sb[:],
                             start=True, stop=False)
            nc.tensor.matmul(out=fc2_ps[:], lhsT=wf2b_sb[:], rhs=fc1b_sb[:],
                             start=False, stop=True)
            nc.scalar.activation(
                out=out_sb[:], in_=fc2_ps[:],
                func=mybir.ActivationFunctionType.Identity, bias=bf2_sb[:],
            )
            nc.sync.dma_start(out=OUT[:], in_=out_sb[:])

    _split_multi_waits(nc)
    return nc


_NC = None


def _get_program():
    global _NC
    if _NC is None:
        _NC = _build_program()
    return _NC


def _prep_inputs(x, edge_index, batch, prot_vec,
                 W1, b1, W2, b2, W3, b3, Wf1, bf1, Wf2, bf2):
    x = np.ascontiguousarray(np.asarray(x, np.float32))
    src = np.asarray(edge_index[0], np.int64)
    dst = np.asarray(edge_index[1], np.int64)

    assert (src // NPG == dst // NPG).all(), "edges must stay within graphs"
    deg = np.bincount(dst, minlength=N_NODES).astype(np.float32) + 1.0
    dinv = (1.0 / np.sqrt(deg)).astype(np.float32)
    coef = (dinv[src] * dinv[dst]).astype(np.float64)

    # AT[g, u, v] = sum of dinv[su]*dinv[sv] over edges (u -> v) + diag dinv^2
    flat = (src * NPG + dst % NPG).astype(np.int64)
    A = np.bincount(flat, weights=coef, minlength=N_NODES * NPG)
    A = A.astype(np.float32).reshape(N_GRAPHS, NPG, NPG)
    di = np.arange(NPG)
    A[:, di, di] += (dinv * dinv).reshape(N_GRAPHS, NPG)

    # per-core block-diagonal layout [GW, COLS_A]
    A_pad = np.zeros((N_CORES, PAD_G, NPG, NPG), np.float32)
    A_pad[:, :GPC] = A.reshape(N_CORES, GPC, NPG, NPG)
    AT_full = np.zeros((N_CORES, GW, GROUPS, GPG, NPG), np.float32)
    Ar = A_pad.reshape(N_CORES, GROUPS, GPG, NPG, NPG)
    for j in range(GPG):
        AT_full[:, NPG * j:NPG * (j + 1), :, j, :] = \
            Ar[:, :, j].transpose(0, 2, 1, 3)
    AT_full = np.ascontiguousarray(
        AT_full.reshape(N_CORES, GW, COLS_A).astype(np.float16))

    # xT with the 128-wide group stride of the H layout
    xm = x.reshape(N_CORES, GPC * NPG, F_IN).transpose(0, 2, 1)  # [c, 13, 6400]
    xT = np.zeros((N_CORES, F_IN, GROUPS, GS), np.float16)
    full = (GPC * NPG) // GW       # 51 full groups
    xT[:, :, :full, :GW] = xm[:, :, :full * GW].reshape(N_CORES, F_IN, full, GW)
    rem = GPC * NPG - full * GW    # 25 leftover cols (graph 255)
    if rem:
        xT[:, :, full, :rem] = xm[:, :, full * GW:]
    xT = np.ascontiguousarray(xT.reshape(N_CORES, F_IN, COLS_H))

    PTm = np.ascontiguousarray(
        np.asarray(prot_vec, np.float16).reshape(N_CORES, GPC, PROT)
        .transpose(0, 2, 1))

    com = {
        "W1": np.ascontiguousarray(np.asarray(W1, np.float16)),
        "W2": np.ascontiguousarray(np.asarray(W2, np.float16)),
        "W3": np.ascontiguousarray(np.asarray(W3, np.float16)),
        "B1": np.asarray(b1, np.float32).reshape(HID, 1),
        "B2": np.asarray(b2, np.float32).reshape(HID, 1),
        "B3": np.asarray(b3, np.float32).reshape(HID, 1),
        "WF1": np.ascontiguousarray(np.asarray(Wf1, np.float16)),
        "BF1": np.asarray(bf1, np.float32).reshape(256, 1),
        "WF2": np.ascontiguousarray(np.asarray(Wf2, np.float16)),
        "BF2": np.asarray(bf2, np.float32).reshape(1, 1),
    }
    in_maps = []
    for c in range(N_CORES):
        m = dict(com)
        m["xT"] = xT[c]
        m["AT"] = AT_full[c]
        m["PT"] = PTm[c]
        in_maps.append(m)
    return in_maps


def _run(inputs, **run_kwargs):
    in_maps = _prep_inputs(**inputs)
    nc = _get_program()
    res = run_bass_kernel_spmd(nc, in_maps, core_ids=list(range(N_CORES)),
                               **run_kwargs)
    out = np.concatenate(
        [r["out"].reshape(GPC, 1) for r in res.results], axis=0)
    return out.astype(np.float32), res


def kernel(**inputs):
    out, _ = _run(inputs)
    return out
